# revision 16
# baseline (speedup 1.0000x reference)
"""Bass/Trainium2 kernel for Bahdanau (additive) attention, 8-core data-parallel.

Reference computation (per batch b):
    att1 = enc[b] @ We                    # [N, A]
    att2 = dec[b] @ Wd + bd               # [A]
    att  = tanh(att1 + att2 + be)         # [N, A]
    s    = att @ Wf (+ bf)                # [N]   (bf dropped: softmax-invariant)
    alpha= softmax(s)                     # [N]
    ctx  = sum_n alpha[n] * enc[b, n]     # [E]

Sharding: batch 128 -> 8 cores x 16. Weights replicated. No collectives.

Per-core layout strategy:
  - att1 computed transposed ([A_chunk=128, n]) with batches processed in
    pairs so the PE moving operand is [128, 392] (>=256 -> float32r full rate).
    lhsT = We blocks (natural layout), moving = encT (PE-transposed enc).
  - att2 add fused into tanh on ScalarE as a per-partition bias.
  - scores via PE: lhsT = Wf chunk [128,1], moving = att tiles.
  - softmax on a single partition per pair (reduce_max/exp+accum/recip).
  - context via PE with a zero-padded block-diagonal alphaT (each batch's
    alpha column placed at 256-aligned row offsets), accumulating all 16
    batches into 4 persistent PSUM banks [16, 512].
"""

import numpy as np

B, N, E = 128, 196, 2048
D, A = 512, 512
NCORES = 8
BL = B // NCORES            # 16 batches per core
PAIRS = BL // 2             # 8
N0 = 128
N1 = N - N0                 # 68
EC = E // 128               # 16
AC = A // 128               # 4
E4 = E // 512               # 4
DC = D // 128               # 4

_CACHE = {}


def _patch_tile_tail_drain(tile):
    """This walrus build rejects >1 sem-wait per instruction. Split extra
    waits onto single-wait NOPs committed just before the instruction, and
    do the same for the TileContext tail drain."""
    import concourse.mybir as mybir
    from concourse.vector_clock import ScopedClock

    if getattr(tile.TileContext, "_tail_drain_patched", False):
        return

    orig_commit = tile.TileContext._commit_instruction

    def _commit_instruction(self, inst, lazy_reg_writes=True):
        si = getattr(inst, "sync_info", None)
        engine = getattr(inst, "engine", None)
        if (
            si is not None
            and si.on_wait
            and len(si.on_wait) > 1
            and engine is not None
            and engine != mybir.EngineType.Unassigned
            and type(inst).__name__.startswith("Inst")
        ):
            waits = list(si.on_wait)
            for i, w in enumerate(waits[:-1]):
                noop = mybir.InstNoOp(
                    name=self.nc.get_next_instruction_name(),
                    sync_info=mybir.SyncInfo(on_wait=[w], on_update=[]),
                    bass_nofuse=True,
                    engine=engine,
                )
                orig_commit(self, noop, lazy_reg_writes=False)
            inst.sync_info = mybir.SyncInfo(
                on_wait=[waits[-1]], on_update=list(si.on_update or [])
            )
        return orig_commit(self, inst, lazy_reg_writes)

    tile.TileContext._commit_instruction = _commit_instruction

    def _drain_and_barrier(self, tick_clock, wait_clock):
        nc = self.nc
        drain_inst = nc.sync.drain()
        wait_clock.add_sem_waits(
            drain_inst.ins, ScopedClock({None: tick_clock.global_clock})
        )
        si = drain_inst.ins.sync_info
        waits = list(si.on_wait or []) if si is not None else []
        if len(waits) > 1:
            drain_inst.ins.sync_info = mybir.SyncInfo(
                on_wait=waits[:1], on_update=list(si.on_update or [])
            )
            for w in waits[1:]:
                d = nc.sync.drain()
                d.ins.sync_info = mybir.SyncInfo(on_wait=[w], on_update=[])
        nc.all_engine_barrier()
        assert self.sems is not None
        popped = nc._tile_sem_poison_stack.pop()
        assert popped is self._sem_poison
        nc.clear_and_free_semaphores(list(self.sems.allocated().values()))
        nc.all_engine_barrier()

    tile.TileContext._drain_and_barrier = _drain_and_barrier
    tile.TileContext._tail_drain_patched = True


def build():
    from contextlib import ExitStack

    import concourse.bass as bass
    import concourse.tile as tile
    from concourse import mybir
    from concourse.bass import ts
    from concourse.masks import make_identity

    _patch_tile_tail_drain(tile)

    f32 = mybir.dt.float32
    f32r = mybir.dt.float32r
    TANH = mybir.ActivationFunctionType.Tanh
    EXP = mybir.ActivationFunctionType.Exp

    nc = bass.Bass(trn_type="TRN2", target_bir_lowering=False, debug=False)
    enc_d = nc.dram_tensor("encoder_out", [BL, N, E], f32, kind="ExternalInput")
    dec_d = nc.dram_tensor("decoder_hidden", [BL, D], f32, kind="ExternalInput")
    we_d = nc.dram_tensor("We", [E, A], f32, kind="ExternalInput")
    be_d = nc.dram_tensor("be", [A], f32, kind="ExternalInput")
    wd_d = nc.dram_tensor("Wd", [D, A], f32, kind="ExternalInput")
    bd_d = nc.dram_tensor("bd", [A], f32, kind="ExternalInput")
    wf_d = nc.dram_tensor("Wf", [A, 1], f32, kind="ExternalInput")
    ctx_d = nc.dram_tensor("context", [BL, E], f32, kind="ExternalOutput")
    alp_d = nc.dram_tensor("alpha", [BL, N, 1], f32, kind="ExternalOutput")

    def r(ap):
        return ap.bitcast(f32r)

    with tile.TileContext(nc) as tc, ExitStack() as ctx:
        consts = ctx.enter_context(tc.tile_pool(name="consts", bufs=1))
        work = ctx.enter_context(tc.tile_pool(name="work", bufs=2))
        psum = ctx.enter_context(tc.tile_pool(name="psum", bufs=1, space="PSUM"))

        # ---- constants / weights ----
        ident = consts.tile([128, 128], f32)
        make_identity(nc, ident[:])
        ident_r = consts.tile([128, 128], f32r)
        nc.vector.tensor_copy(ident_r[:], ident[:])

        we_sb = consts.tile([128, EC, A], f32r)
        for c in range(EC):
            nc.sync.dma_start(we_sb[:, c, :], we_d.ap()[ts(c, 128), :].bitcast(f32r))
        wd_sb = consts.tile([128, DC, A], f32)
        for c in range(DC):
            nc.sync.dma_start(wd_sb[:, c, :], wd_d.ap()[ts(c, 128), :])
        wf_sb = consts.tile([128, AC], f32r)
        nc.sync.dma_start(
            wf_sb[:], wf_d.ap().rearrange("(c p) o -> p (c o)", p=128).bitcast(f32r)
        )
        be_sb = consts.tile([128, AC], f32)
        bd_sb = consts.tile([128, AC], f32)
        nc.sync.dma_start(be_sb[:], be_d.ap().rearrange("(c p) -> p c", p=128))
        nc.sync.dma_start(bd_sb[:], bd_d.ap().rearrange("(c p) -> p c", p=128))
        bdbe = consts.tile([128, AC], f32)
        nc.vector.tensor_add(bdbe[:], be_sb[:], bd_sb[:])

        dec_sb = consts.tile([BL, D], f32)
        nc.sync.dma_start(dec_sb[:], dec_d.ap())

        # decT [d, b] via PE transposes
        decT_sb = consts.tile([128, DC, BL], f32)
        for c in range(DC):
            ps_tp = psum.tile([128, 392], f32, tag="tp", bufs=2)
            nc.tensor.transpose(
                ps_tp[:, 0:BL], dec_sb[:, ts(c, 128)], ident[0:BL, 0:BL]
            )
            nc.vector.tensor_copy(decT_sb[:, c, :], ps_tp[:, 0:BL])

        # att2T [a, b] = Wd.T @ decT (+ bd + be), f32 exact
        att2T_sb = consts.tile([128, AC, BL], f32)
        for ca in range(AC):
            ps_a2 = psum.tile([128, 392], f32, tag="tp", bufs=2)
            for cd in range(DC):
                nc.tensor.matmul(
                    ps_a2[:, 0:BL],
                    wd_sb[:, cd, ts(ca, 128)],
                    decT_sb[:, cd, :],
                    start=(cd == 0),
                    stop=(cd == DC - 1),
                )
            nc.scalar.add(att2T_sb[:, ca, :], ps_a2[:, 0:BL], bdbe[:, ca : ca + 1])

        # block-diagonal padded alphaT: row 256*b + n holds alpha[b, n] in col b
        alphaT_pad = consts.tile([128, 2 * BL, BL], f32r)
        nc.gpsimd.memset(alphaT_pad[:].bitcast(f32), 0.0)

        # persistent context accumulators: 4 banks of [16, 512]
        ctx_ps = psum.tile([BL, E4, 512], f32, tag="ctx", bufs=1)

        def copy_op(i, out, in_):
            # split bulk PSUM->SBUF evacuation across DVE and ACT
            if i % 2 == 0:
                nc.vector.tensor_copy(out, in_)
            else:
                nc.scalar.copy(out, in_)

        for p in range(PAIRS):
            b0, b1 = 2 * p, 2 * p + 1

            nat0 = work.tile([128, 2, E], f32r, tag="nat0", bufs=2)
            nat1 = work.tile([N1, 2, E], f32r, tag="nat1", bufs=2)
            for j, b in enumerate((b0, b1)):
                nc.sync.dma_start(nat0[:, j, :], enc_d.ap()[b, 0:N0, :].bitcast(f32r))
                nc.sync.dma_start(nat1[:, j, :], enc_d.ap()[b, N0:N, :].bitcast(f32r))

            # transpose enc into [e, (b-pair n)] tiles
            encT = work.tile([128, EC, 392], f32r, tag="encT", bufs=2)
            for ce in range(EC):
                ps_t = psum.tile([128, 392], f32r, tag="tp", bufs=2)
                for j in range(2):
                    off = 196 * j
                    nc.tensor.transpose(
                        ps_t[:, off : off + N0],
                        nat0[:, j, ts(ce, 128)],
                        ident_r[:, :],
                    )
                    nc.tensor.transpose(
                        ps_t[:, off + N0 : off + N],
                        nat1[:, j, ts(ce, 128)],
                        ident_r[0:N1, 0:N1],
                    )
                copy_op(ce, encT[:, ce, :], ps_t[:])

            # att1T = (enc @ We).T per pair + fused tanh(. + att2)
            att = work.tile([128, AC, 392], f32r, tag="att", bufs=1)
            for ca in range(AC):
                ps_a = psum.tile([128, 392], f32, tag="att1", bufs=2)
                for ce in range(EC):
                    nc.tensor.matmul(
                        ps_a[:],
                        r(we_sb[:, ce, ts(ca, 128)]),
                        r(encT[:, ce, :]),
                        start=(ce == 0),
                        stop=(ce == EC - 1),
                    )
                nc.scalar.activation(
                    att[:, ca, 0:196], ps_a[:, 0:196], TANH,
                    bias=att2T_sb[:, ca, b0 : b0 + 1],
                )
                nc.scalar.activation(
                    att[:, ca, 196:392], ps_a[:, 196:392], TANH,
                    bias=att2T_sb[:, ca, b1 : b1 + 1],
                )

            # scores [1, 392] = Wf.T @ att
            ps_s = psum.tile([128, 392], f32, tag="tp", bufs=2)
            for ca in range(AC):
                nc.tensor.matmul(
                    ps_s[0:1, :],
                    r(wf_sb[:, ca : ca + 1]),
                    r(att[:, ca, :]),
                    start=(ca == 0),
                    stop=(ca == AC - 1),
                )
            sc_sb = work.tile([1, 392], f32, tag="sc", bufs=1)
            nc.vector.tensor_copy(sc_sb[:], ps_s[0:1, :])

            # softmax over n (196) for each half, on partition 0
            mx = work.tile([1, 2], f32, tag="mx", bufs=2)
            sm = work.tile([1, 2], f32, tag="sm", bufs=2)
            rs = work.tile([1, 2], f32, tag="rs", bufs=2)
            nc.vector.reduce_max(
                mx[:],
                sc_sb[:].rearrange("p (h n) -> p h n", h=2),
                axis=mybir.AxisListType.X,
                negate=True,
            )
            al = work.tile([1, 392], f32, tag="al", bufs=2)
            for h in range(2):
                nc.scalar.activation(
                    al[0:1, ts(h, 196)], sc_sb[0:1, ts(h, 196)], EXP,
                    bias=mx[0:1, h : h + 1],
                    accum_out=sm[0:1, h : h + 1],
                )
            nc.vector.reciprocal(rs[:], sm[:])
            for h in range(2):
                nc.vector.tensor_scalar_mul(
                    al[0:1, ts(h, 196)], al[0:1, ts(h, 196)], rs[0:1, h : h + 1]
                )

            # alpha out
            nc.gpsimd.dma_start(
                alp_d.ap()[b0 : b0 + 2].rearrange("b n o -> o (b n)"), al[:]
            )

            # place alpha columns into the block-diagonal alphaT_pad
            ps_al = psum.tile([128, 392], f32, tag="tp", bufs=2)
            for j, b in enumerate((b0, b1)):
                off = 196 * j
                nc.tensor.transpose(
                    ps_al[:, 2 * j : 2 * j + 1],
                    al[0:1, off : off + N0],
                    ident[0:1, 0:1],
                )
                nc.tensor.transpose(
                    ps_al[0:N1, 2 * j + 1 : 2 * j + 2],
                    al[0:1, off + N0 : off + N],
                    ident[0:1, 0:1],
                )
                nc.vector.tensor_copy(
                    alphaT_pad[:, 2 * b, b : b + 1], ps_al[:, 2 * j : 2 * j + 1]
                )
                nc.vector.tensor_copy(
                    alphaT_pad[0:N1, 2 * b + 1, b : b + 1],
                    ps_al[0:N1, 2 * j + 1 : 2 * j + 2],
                )

            # context accumulation: 2 matmuls per (b, e512-chunk)
            for j, b in enumerate((b0, b1)):
                for e4 in range(E4):
                    nc.tensor.matmul(
                        ctx_ps[:, e4, :],
                        r(alphaT_pad[:, 2 * b, :]),
                        r(nat0[:, j, ts(e4, 512)]),
                        start=(p == 0 and j == 0),
                        stop=False,
                        skip_group_check=True,
                    )
                    nc.tensor.matmul(
                        ctx_ps[:, e4, :],
                        r(alphaT_pad[0:N1, 2 * b + 1, :]),
                        r(nat1[:, j, ts(e4, 512)]),
                        start=False,
                        stop=(p == PAIRS - 1 and j == 1),
                        skip_group_check=True,
                    )

        # evacuate context and store
        ctx_sb = consts.tile([BL, E], f32)
        for e4 in range(E4):
            copy_op(e4, ctx_sb[:, ts(e4, 512)], ctx_ps[:, e4, :])
        nc.gpsimd.dma_start(ctx_d.ap(), ctx_sb[:])

    return nc


def _get_nc():
    if "nc" not in _CACHE:
        _CACHE["nc"] = build()
    return _CACHE["nc"]


def _run(in_maps, trace=False, tmpdir=None):
    from concourse.bass_utils import run_bass_kernel_spmd

    nc = _get_nc()
    return run_bass_kernel_spmd(
        nc, in_maps, core_ids=list(range(NCORES)), trace=trace, tmpdir=tmpdir
    )


def make_in_maps(encoder_out, decoder_hidden, We, be, Wd, bd, Wf, bf=None):
    enc = np.ascontiguousarray(np.asarray(encoder_out, dtype=np.float32))
    dec = np.ascontiguousarray(np.asarray(decoder_hidden, dtype=np.float32))
    We = np.ascontiguousarray(np.asarray(We, dtype=np.float32))
    be = np.ascontiguousarray(np.asarray(be, dtype=np.float32))
    Wd = np.ascontiguousarray(np.asarray(Wd, dtype=np.float32))
    bd = np.ascontiguousarray(np.asarray(bd, dtype=np.float32))
    Wf = np.ascontiguousarray(np.asarray(Wf, dtype=np.float32))
    in_maps = []
    for i in range(NCORES):
        sl = slice(i * BL, (i + 1) * BL)
        in_maps.append(
            dict(
                encoder_out=enc[sl],
                decoder_hidden=dec[sl],
                We=We,
                be=be,
                Wd=Wd,
                bd=bd,
                Wf=Wf,
            )
        )
    return in_maps


def gather(results):
    context = np.concatenate([results[i]["context"] for i in range(NCORES)], axis=0)
    alpha = np.concatenate([results[i]["alpha"] for i in range(NCORES)], axis=0)
    return context, alpha


def kernel(encoder_out, decoder_hidden, We, be, Wd, bd, Wf, bf):
    in_maps = make_in_maps(encoder_out, decoder_hidden, We, be, Wd, bd, Wf, bf)
    res = _run(in_maps, trace=False)
    _CACHE["last_results"] = res
    return gather(res.results)
